# revision 7
# baseline (speedup 1.0000x reference)
"""Trainium2 Bass kernel for MixedCausalAttention (16 heads, d=1024, L_S=4096, L_NS=64).

Sharding: tensor-parallel over heads - 2 heads per core x 8 cores.
Each core computes qkv projections (shared W_S for S tokens, per-token W_NS for
NS tokens) for its 2 heads, causal attention, and a partial W_out product over
its 128 output feature rows. The host sums the 8 partial (2112, 1024) outputs.

v2 layout/dtype strategy:
- bf16 on the whole matmul path (x, W_S, Q/K/V, exp'd scores, W_out); PSUM
  accumulation stays fp32. Measured end-to-end rel err ~4e-3 vs the 2e-2 gate.
- The dominant W_NS stream (805MB fp32 total) is fp8_e4m3 (scaled x64), and the
  per-token projection contracts 256 rows/pass via MatmulPerfMode.DoubleRow
  (2 fp8 MACs/cell/cycle). Only the last 64 output rows see fp8 error (S
  queries never attend to NS keys); total rel err ~6.5e-3.
- Scores matmuls have K=64: the two heads ride in row-groups 0-1/2-3 of the PE
  via implicit tile_position=(0,0)/(64,0), so they execute concurrently.
- V is produced in natural [seq, dh] layout directly (x-chunk stationary
  instead of W stationary), eliminating the V^T transpose pass.
- Softmax denominator comes from an extra all-ones column in the attn@V
  stationary (row 64 of the PSUM accumulator); normalization multiplies the
  broadcast reciprocal on DVE; the two heads' normalized outputs are packed
  into one [128, q] tile (partition move via tiny SWDGE DMA) so W_out is a
  single K=128 matmul.
- Causal masking: affine_select staircase on diagonal-crossing tiles only;
  fully-masked key-chunks are skipped, and the leading fully-masked columns of
  diagonal chunks are trimmed (c0) from matmul/exp/select/attn@V.
"""

import os
import sys
import math
from concurrent.futures import ThreadPoolExecutor

for _p in ("/opt/trn_rl_repo", "/root/.axon_site/_ro/trn_rl_repo"):
    if os.path.isdir(_p) and _p not in sys.path:
        sys.path.insert(0, _p)

import numpy as np
import ml_dtypes

import concourse.bass as bass
import concourse.mybir as mybir
import concourse.tile as tile
from concourse import bacc
from concourse.bass_utils import run_bass_kernel_spmd

F32 = mybir.dt.float32
BF16 = mybir.dt.bfloat16
F8 = mybir.dt.float8e4
DR = mybir.MatmulPerfMode.DoubleRow

N_CORES = 8
D = 1024
H = 16
DH = 64
HPC = H // N_CORES          # heads per core = 2
O3 = 3 * DH * HPC           # 384 qkv output cols per core
LNS = 64
LS = 4096
QS = 2048                   # query_start
LQ = LS - QS + LNS          # 2112 queries
NCH = D // 128              # 8 contraction chunks
ST = 512                    # s-tile width for projections
QT = 512                    # q-tile width for attention
SCALE = DH ** -0.5
XNS_SC = 8.0                # fp8 pre-scales (undone after the NS matmul)
WNS_SC = 64.0


def build_program(repeat=1):
    nc = bacc.Bacc("TRN2", target_bir_lowering=False, debug=False,
                   num_devices=N_CORES)

    xt_d = nc.dram_tensor("xt", [128, NCH, LS], BF16, kind="ExternalInput")
    xnst_d = nc.dram_tensor("xnst", [128, NCH, LNS], F8, kind="ExternalInput")
    ws_d = nc.dram_tensor("ws", [128, NCH, O3], BF16, kind="ExternalInput")
    wns_d = nc.dram_tensor("wns", [LNS, 128, NCH, O3], F8, kind="ExternalInput")
    wout_d = nc.dram_tensor("wout", [128, D], BF16, kind="ExternalInput")
    vones_d = nc.dram_tensor("vones", [128, 64], BF16, kind="ExternalInput")
    ident_d = nc.dram_tensor("ident", [64, 64], BF16, kind="ExternalInput")
    o_d = nc.dram_tensor("o", [LQ, D], F32, kind="ExternalOutput")

    n_kc_s = LS // 128       # 32 S key chunks
    n_kc = n_kc_s + 1        # + NS chunk
    lqs = LS - QS            # 2048 S-query columns

    with tile.TileContext(nc) as tc:
      for _rep in range(repeat):
        import contextlib
        ctx = contextlib.ExitStack()
        with ctx:
            const = ctx.enter_context(tc.tile_pool(name="const", bufs=1))
            store = ctx.enter_context(tc.tile_pool(name="store", bufs=1))

            # --- constants ---
            ws_sb = const.tile([128, NCH, O3], BF16)
            nc.sync.dma_start(out=ws_sb, in_=ws_d.ap())
            xpool = ctx.enter_context(tc.tile_pool(name="xpool", bufs=2))
            xt0_t = xpool.tile([128, NCH, ST], BF16, tag="xt", name="xt0_t")
            nc.sync.dma_start(out=xt0_t, in_=xt_d.ap()[:, :, 0:ST])
            xnst_sb = const.tile([128, NCH, LNS], F8)
            nc.sync.dma_start(out=xnst_sb, in_=xnst_d.ap())
            wout_sb = const.tile([128, D], BF16)
            nc.sync.dma_start(out=wout_sb, in_=wout_d.ap())
            ident_sb = const.tile([64, 64], BF16)
            nc.sync.dma_start(out=ident_sb, in_=ident_d.ap())
            ones_sb = const.tile([65, 64], BF16)
            nc.sync.dma_start(out=ones_sb, in_=vones_d.ap()[0:65, 0:64])

            # --- persistent activation storage ---
            qt_s = store.tile([128, lqs], BF16)     # Q^T, S part (h0 rows 0-63, h1 64-127)
            qt_ns = store.tile([128, LNS], BF16)    # Q^T, NS part
            kt_s = store.tile([128, LS], BF16)      # K^T, S part
            kt_ns = store.tile([128, LNS], BF16)    # K^T, NS part
            v_s = [store.tile([128, n_kc_s, 65], BF16, name=f"v_s{h}")
                   for h in range(2)]
            v_ns = [store.tile([64, 65], BF16, name=f"v_ns{h}") for h in range(2)]
            stg_all = store.tile([1, LNS, O3], BF16)  # NS qkv rows on partition 0
            qkvns_sb = store.tile([LNS, O3], BF16)    # natural-layout NS qkv rows

            # ones columns for the denominator trick
            for h in range(2):
                nc.sync.dma_start(out=v_s[h][:, :, 64:65],
                                  in_=vones_d.ap()[:, 0:n_kc_s])
                nc.sync.dma_start(out=v_ns[h][:, 64:65],
                                  in_=vones_d.ap()[0:64, 0:1])

            # ---------------- NS-token projections (emitted interleaved) ----
            wnspool = ctx.enter_context(tc.tile_pool(name="wnspool", bufs=4))
            psNS = ctx.enter_context(tc.tile_pool(name="psNS", bufs=2, space="PSUM"))

            def ns_emitter():
                inv = 1.0 / (XNS_SC * WNS_SC)
                for n in range(LNS):
                    wns_t = wnspool.tile([128, NCH, O3], F8, tag="wns")
                    nc.sync.dma_start(out=wns_t, in_=wns_d.ap()[n])
                    psn = psNS.tile([1, O3], F32, tag="psNS")
                    for kp in range(NCH // 2):
                        nc.tensor.matmul(
                            psn[:, :],
                            lhsT=xnst_sb[:, 2 * kp:2 * kp + 2, n:n + 1],
                            rhs=wns_t[:, 2 * kp:2 * kp + 2, :],
                            start=(kp == 0), stop=(kp == NCH // 2 - 1),
                            perf_mode=DR)
                    # DVE unscale-copy (keeps ACT exclusively on Exp — no
                    # activation-table reloads)
                    nc.vector.tensor_scalar_mul(
                        out=stg_all[0:1, n, :], in0=psn[:, :], scalar1=inv)
                    yield
                # finalize: scatter rows to natural layout, then Q/K transposes
                nc.gpsimd.dma_start(out=qkvns_sb[:, :], in_=stg_all[0:1, :, :])
                for part, dest in ((0, qt_ns), (1, kt_ns)):
                    pst2 = psNS.tile([128, 64], BF16, tag="psNS", name="pst2")
                    nc.tensor.transpose(
                        pst2[:, :], qkvns_sb[0:64, part * 128:(part + 1) * 128],
                        ident_sb[:, :])
                    nc.vector.tensor_copy(out=dest[:, :], in_=pst2[:, :])
                for h in range(2):
                    nc.vector.tensor_copy(
                        out=v_ns[h][0:64, 0:64],
                        in_=qkvns_sb[0:64, 256 + h * 64:256 + (h + 1) * 64])
                while True:
                    yield

            ns_gen = ns_emitter()
            ns_left = LNS + 1  # tokens + finalize step

            # ---------------- stage A: S-token projections ----------------
            # K^T/Q^T via W-stationary matmuls; V natural via x-stationary.
            with tc.tile_pool(name="psA", bufs=2, space="PSUM") as psA, \
                 tc.tile_pool(name="psV", bufs=2, space="PSUM") as psV:
                for st in range(LS // ST):
                    s0 = st * ST
                    if st == 0:
                        xt_t = xt0_t
                    else:
                        xt_t = xpool.tile([128, NCH, ST], BF16, tag="xt",
                                          name="xt_t")
                        nc.sync.dma_start(out=xt_t, in_=xt_d.ap()[:, :, s0:s0 + ST])
                    jobs = [(1, kt_s, s0)]
                    if s0 >= QS:
                        jobs.append((0, qt_s, s0 - QS))
                    for mi, dest, dcol in jobs:
                        ps = psA.tile([128, ST], F32, tag="psA")
                        for ci in range(NCH):
                            nc.tensor.matmul(
                                ps[:, :],
                                lhsT=ws_sb[:, ci, mi * 128:(mi + 1) * 128],
                                rhs=xt_t[:, ci, :],
                                start=(ci == 0), stop=(ci == NCH - 1))
                        nc.vector.tensor_copy(out=dest[:, dcol:dcol + ST], in_=ps[:, :])
                    # V natural: x chunk stationary, W_V cols moving
                    for sc in range(ST // 128):
                        kcidx = (s0 + sc * 128) // 128
                        psv = psV.tile([128, 128], F32, tag="psV")
                        for ci in range(NCH):
                            nc.tensor.matmul(
                                psv[:, :],
                                lhsT=xt_t[:, ci, sc * 128:(sc + 1) * 128],
                                rhs=ws_sb[:, ci, 256:384],
                                start=(ci == 0), stop=(ci == NCH - 1))
                        for h in range(2):
                            nc.vector.tensor_copy(
                                out=v_s[h][:, kcidx, 0:64],
                                in_=psv[:, h * 64:(h + 1) * 64])
                    if ns_left > 0:
                        next(ns_gen)
                        ns_left -= 1
                    if ns_left > 0 and st % 2 == 1:
                        next(ns_gen)
                        ns_left -= 1

            # ---------------- main attention loop ----------------
            expool = ctx.enter_context(tc.tile_pool(name="expool", bufs=6))
            recpool = ctx.enter_context(tc.tile_pool(name="recpool", bufs=2))
            bcpool = ctx.enter_context(tc.tile_pool(name="bcpool", bufs=2))
            avpool = ctx.enter_context(tc.tile_pool(name="avpool", bufs=2))
            av1pool = ctx.enter_context(tc.tile_pool(name="av1pool", bufs=2))
            outpool = ctx.enter_context(tc.tile_pool(name="outpool", bufs=2))
            psS = ctx.enter_context(tc.tile_pool(name="psS", bufs=3, space="PSUM"))
            psAV = ctx.enter_context(tc.tile_pool(name="psAV", bufs=2, space="PSUM"))
            psMisc = ctx.enter_context(tc.tile_pool(name="psMisc", bufs=1, space="PSUM"))

            q_tiles = [(q0, min(QT, LQ - q0)) for q0 in range(0, LQ, QT)]
            for qt_i, (q0, qw) in enumerate(q_tiles):
                kc_count = min((QS + q0 + qw - 1) // 128 + 1, n_kc)
                is_last_qt = (q0 >= lqs)
                if is_last_qt:
                    # drain any remaining NS emission before NS data is needed
                    while ns_left > 0:
                        next(ns_gen)
                        ns_left -= 1
                ps_av = [psAV.tile([65, QT], F32, tag="psAV", name=f"ps_av{h}")
                         for h in range(2)]
                for kc in range(kc_count):
                    is_ns_chunk = (kc == n_kc_s)
                    kw = LNS if is_ns_chunk else 128
                    # trim leading fully-masked query columns of this chunk
                    c0 = 0 if kc == 0 else max(0, 128 * kc - QS - q0)
                    assert c0 < qw
                    ps_s = []
                    for h in range(2):
                        hs = slice(h * 64, h * 64 + 64)
                        if is_ns_chunk:
                            k_src = kt_ns[hs, 0:kw]
                        else:
                            k_src = kt_s[hs, kc * 128:kc * 128 + kw]
                        if is_last_qt:
                            q_src = qt_ns[hs, q0 - lqs + c0:q0 - lqs + qw]
                        else:
                            q_src = qt_s[hs, q0 + c0:q0 + qw]
                        p = psS.tile([128, QT], F32, tag="psS", name=f"ps_s{h}")
                        nc.tensor.matmul(p[0:kw, c0:qw], lhsT=k_src, rhs=q_src,
                                         start=True, stop=True)
                        ps_s.append(p)
                    exs = []
                    for h in range(2):
                        ex = expool.tile([128, QT], BF16, tag="exp")
                        nc.scalar.activation(
                            out=ex[0:kw, c0:qw], in_=ps_s[h][0:kw, c0:qw],
                            func=mybir.ActivationFunctionType.Exp, scale=SCALE)
                        if 128 * kc + kw - 1 > QS + q0 + c0:
                            # causal staircase: keep iff q_pos - k_pos >= 0
                            nc.gpsimd.affine_select(
                                out=ex[0:kw, c0:qw], in_=ex[0:kw, c0:qw],
                                compare_op=mybir.AluOpType.is_ge, fill=0.0,
                                base=QS + q0 + c0 - 128 * kc,
                                channel_multiplier=-1,
                                pattern=[[1, qw - c0]])
                        exs.append(ex)
                    for h in range(2):
                        v_src = v_ns[h][0:kw, 0:65] if is_ns_chunk \
                            else v_s[h][0:kw, kc, 0:65]
                        nc.tensor.matmul(ps_av[h][0:65, c0:qw], lhsT=v_src,
                                         rhs=exs[h][0:kw, c0:qw],
                                         start=(kc == 0), stop=(kc == kc_count - 1))
                    if ns_left > 0:
                        next(ns_gen)
                        ns_left -= 1

                # normalize: per-head reciprocal of the ones-row sums, broadcast
                # across partitions via a K=1 PE matmul, multiplied in on DVE.
                # Both heads' normalized outputs pack into av_pk [128, qw].
                av_pk = avpool.tile([128, QT], BF16, tag="avpk")
                for h in range(2):
                    avu = recpool.tile([65, QT], F32, tag=f"avu{h}", name=f"avu{h}")
                    nc.vector.tensor_copy(out=avu[0:65, 0:qw], in_=ps_av[h][0:65, 0:qw])
                    # bf16 so the broadcast matmul streams 1 cycle/row (walrus
                    # rejects f32r x bf16 mixes); ~1e-3 on the normalization
                    rc = recpool.tile([65, QT], BF16, tag="recip")
                    with nc.allow_low_precision(reason="bf16 recip broadcast"):
                        nc.vector.reciprocal(out=rc[64:65, 0:qw], in_=avu[64:65, 0:qw])
                    pbc = psMisc.tile([128, 512], F32, tag="psMisc", name="pbc")
                    nc.tensor.matmul(pbc[0:64, 0:qw], lhsT=ones_sb[64:65, 0:64],
                                     rhs=rc[64:65, 0:qw], start=True, stop=True)
                    bc = bcpool.tile([64, QT], F32, tag="bcast")
                    nc.vector.tensor_copy(out=bc[0:64, 0:qw], in_=pbc[0:64, 0:qw])
                    if h == 0:
                        nc.vector.tensor_mul(av_pk[0:64, 0:qw], avu[0:64, 0:qw],
                                             bc[0:64, 0:qw])
                    else:
                        av1 = av1pool.tile([64, QT], BF16, tag="av1")
                        nc.vector.tensor_mul(av1[0:64, 0:qw], avu[0:64, 0:qw],
                                             bc[0:64, 0:qw])
                        # partition move rows 0-63 -> 64-127 (engines can't)
                        nc.gpsimd.dma_start(out=av_pk[64:128, 0:qw],
                                            in_=av1[0:64, 0:qw])

                # W_out partial: out[q, :] = av_pk[:, q].T @ wout  (K=128)
                for qs in range(math.ceil(qw / 128)):
                    qsw = min(128, qw - qs * 128)
                    ot = outpool.tile([128, D], F32, tag="out")
                    for e in range(2):
                        po = psMisc.tile([128, 512], F32, tag="psMisc", name="po")
                        nc.tensor.matmul(
                            po[0:qsw, :],
                            lhsT=av_pk[:, qs * 128:qs * 128 + qsw],
                            rhs=wout_sb[:, e * 512:(e + 1) * 512],
                            start=True, stop=True)
                        nc.vector.tensor_copy(out=ot[0:qsw, e * 512:(e + 1) * 512],
                                              in_=po[0:qsw, :])
                    nc.sync.dma_start(
                        out=o_d.ap()[q0 + qs * 128:q0 + qs * 128 + qsw, :],
                        in_=ot[0:qsw, :])

    nc.compile()
    return nc


_NC_CACHE = {}


def _get_program():
    if "nc" not in _NC_CACHE:
        _NC_CACHE["nc"] = build_program()
    return _NC_CACHE["nc"]


def _prep_core(c, xt, W_S, W_NS, W_out):
    """Host-side shard prep for core c (heads 2c, 2c+1)."""
    h0 = 2 * c * DH
    cols = np.r_[h0:h0 + HPC * DH,
                 D + h0:D + h0 + HPC * DH,
                 2 * D + h0:2 * D + h0 + HPC * DH]
    ws = W_S[:, cols].reshape(NCH, 128, O3).transpose(1, 0, 2)
    ws = np.ascontiguousarray(ws.astype(ml_dtypes.bfloat16))
    wns = np.clip(W_NS[:, :, cols] * WNS_SC, -240.0, 240.0)
    wns = wns.reshape(LNS, NCH, 128, O3).transpose(0, 2, 1, 3)
    wns = np.ascontiguousarray(wns.astype(ml_dtypes.float8_e4m3))
    wout = np.ascontiguousarray(
        W_out[h0:h0 + 2 * DH].astype(ml_dtypes.bfloat16))
    return {"xt": xt, "xnst": _prep_core.xnst, "ws": ws, "wns": wns,
            "wout": wout,
            "vones": np.ones((128, 64), dtype=ml_dtypes.bfloat16),
            "ident": np.eye(64, dtype=ml_dtypes.bfloat16)}


def kernel(x, W_S, W_NS, W_out, L_S=None, query_start=None, **_unused):
    x = np.asarray(x, dtype=np.float32)
    W_S = np.asarray(W_S, dtype=np.float32)
    W_NS = np.asarray(W_NS, dtype=np.float32)
    W_out = np.asarray(W_out, dtype=np.float32)
    if L_S is not None:
        assert int(L_S) == LS, f"kernel hardcodes L_S={LS}, got {int(L_S)}"
    if query_start is not None:
        assert int(query_start) == QS, \
            f"kernel hardcodes query_start={QS}, got {int(query_start)}"
    assert x.shape == (1, LS + LNS, D)

    nc = _get_program()

    xs = x[0]                                         # (4160, 1024)
    x2 = xs[:LS].T.reshape(NCH, 128, LS).transpose(1, 0, 2)
    xt = np.ascontiguousarray(x2.astype(ml_dtypes.bfloat16))  # (128, 8, 4096)
    xnst = np.clip(xs[LS:].T * XNS_SC, -240.0, 240.0)
    xnst = xnst.reshape(NCH, 128, LNS).transpose(1, 0, 2)
    _prep_core.xnst = np.ascontiguousarray(xnst.astype(ml_dtypes.float8_e4m3))

    with ThreadPoolExecutor(max_workers=N_CORES) as ex:
        in_maps = list(ex.map(lambda c: _prep_core(c, xt, W_S, W_NS, W_out),
                              range(N_CORES)))

    res = None
    for attempt in range(3):
        try:
            res = run_bass_kernel_spmd(nc, in_maps, list(range(N_CORES)))
            break
        except Exception:
            if attempt == 2:
                raise
            # transient device wedges (NRT_EXEC_UNIT_UNRECOVERABLE) have been
            # observed to clear after the terminal resets the session
            import time
            time.sleep(100)
    out = np.zeros((LQ, D), dtype=np.float32)
    for r in res.results:
        out += r["o"]
    return out.reshape(1, LQ, D)


if __name__ == "__main__":
    rng = np.random.default_rng(0)
    ins = {
        "x": rng.standard_normal((1, LS + LNS, D), dtype=np.float32),
        "W_S": rng.standard_normal((D, 3 * D), dtype=np.float32) * 0.02,
        "W_NS": rng.standard_normal((LNS, D, 3 * D), dtype=np.float32) * 0.02,
        "W_out": rng.standard_normal((D, D), dtype=np.float32) * 0.03,
        "L_S": LS, "query_start": QS,
    }
    out = kernel(**ins)
    print("kernel out shape:", out.shape, "finite:", np.isfinite(out).all())
